# revision 41
# baseline (speedup 1.0000x reference)
# Multi-head causal self-attention (B=2, S=2048, D=768, H=12) on 8 NeuronCores.
#
# Sharding: (batch, head-group) across cores. Core c handles batch c//4 and
# heads 3*(c%4) .. 3*(c%4)+2. Each core computes its heads' Q/K/V projections
# (column-sharded), the causal attention for those heads, and a row-sharded
# partial of the output projection. Host sums the 4 partials per batch + bo.
#
# Engine plan (v4):
#  - PE batches work by tile-size mode so the array never mode-switch-drains
#    mid-stream: (128,128) for QK/V projections and AV (M=65, ones column
#    accumulates the softmax denominator); (64,128) for scores (heads 0,1
#    pair-stacked on partitions and issued to row tiles T0/T8 which run
#    concurrently; head 2 on T0) and the K=64 output projection.
#  - Attention chunks are processed in DESCENDING size order (ic3..ic0) with
#    all QK projections hoisted to the front: the long exp streams start
#    early and the kernel tail is the smallest chunk.
#  - ACT runs only exp. 1/Z = DVE reciprocal, partition-broadcast via a DRAM
#    round-trip DMA; the normalize multiplies are deferred in the DVE queue
#    so the DMA latency never blocks projection evacuations.
#  - GPSIMD: causal-mask multiplies. Output DMA'd in bf16; host sums in fp32.

import sys

import ml_dtypes
import numpy as np

sys.path.insert(0, "/opt/trn_rl_repo")

import concourse.bass as bass  # noqa: E402
import concourse.mybir as mybir  # noqa: E402
import concourse.tile as tile  # noqa: E402
from concourse.bass import ts  # noqa: E402
from concourse.bass_utils import run_bass_kernel_spmd  # noqa: E402

F32 = mybir.dt.float32
BF16 = mybir.dt.bfloat16
AF = mybir.ActivationFunctionType
MUL = mybir.AluOpType.mult
ADD = mybir.AluOpType.add
NPBF16 = ml_dtypes.bfloat16

B, S, D, H, HD = 2, 2048, 768, 12, 64
HPC = 3               # heads per core
DQK = 2 * HPC * HD    # 384
DV = HPC * HD         # 192
P = 128
IC = S // 512         # 4 query chunks of 512
KC = D // P           # 6 contraction chunks
NIO = S // P          # 16 token chunks of 128


def _split_excess_waits(nc, max_waits=1):
    # walrus in this env rejects instructions carrying more than ~1-2
    # sync-waits. Move excess waits onto preceding same-engine nops.
    n_split = 0
    for func in nc.m.functions:
        for blk in func.blocks:
            insts = blk.instructions
            out = []
            changed = False
            for inst in insts:
                si = inst.sync_info
                waits = list(si.on_wait) if si and si.on_wait else []
                if len(waits) > max_waits:
                    changed = True
                    for j, w in enumerate(waits[:-max_waits]):
                        out.append(
                            mybir.InstNoOp(
                                name=f"{inst.name}-wsplit{j}",
                                engine=inst.engine,
                                ins=[],
                                outs=[],
                                sync_info=mybir.SyncInfo(
                                    on_wait=[w], on_update=[]
                                ),
                            )
                        )
                        n_split += 1
                    inst.sync_info = mybir.SyncInfo(
                        on_wait=waits[-max_waits:],
                        on_update=list(si.on_update) if si.on_update else [],
                    )
                out.append(inst)
            if changed:
                blk.instructions = out
    return n_split


def _build_module():
    nc = bass.Bass()
    xt_d = nc.dram_tensor("xt", [D, S], BF16, kind="ExternalInput")
    wqk_d = nc.dram_tensor("wqk", [D, DQK], BF16, kind="ExternalInput")
    bqk_d = nc.dram_tensor("bqk", [P, HPC], F32, kind="ExternalInput")
    wv_d = nc.dram_tensor("wv", [D, DV], BF16, kind="ExternalInput")
    wos_d = nc.dram_tensor("wos", [HD, HPC, D], BF16, kind="ExternalInput")
    mask_d = nc.dram_tensor("mask", [P, 4, 2, 512], BF16, kind="ExternalInput")
    out_d = nc.dram_tensor("out", [S, D], BF16, kind="ExternalOutput")
    scratch_d = nc.dram_tensor("scratch", [HD + 1, 512], F32)

    with tile.TileContext(nc) as tc:
        with (
            tc.tile_pool(name="const", bufs=1) as cp,
            tc.tile_pool(name="exp", bufs=40) as exp_p,
            tc.tile_pool(name="zr", bufs=2) as zr_p,
            tc.tile_pool(name="zd", bufs=12, space="DRAM") as zd_p,
            tc.tile_pool(name="outp", bufs=2) as op,
            tc.tile_pool(name="proj", bufs=2, space="PSUM") as proj_p,
            tc.tile_pool(name="scps", bufs=2, space="PSUM") as sc_p,
            tc.tile_pool(name="avps", bufs=2, space="PSUM") as av_p,
        ):
            # ---- PE warm-up source via DVE memset (gpsimd starts slowly) ----
            warm_src = cp.tile([P, 520], BF16)
            nc.vector.memset(warm_src, 1.0)

            # ---- resident SBUF tensors ----
            # DMA triggers are dispatch-serialized (~0.6us each) on the
            # issuing engine's queue: keep first-needed loads on Sync in
            # need-order and push the bulk of xt onto the idle GpSimd queue.
            wqk_sb = cp.tile([P, KC, DQK], BF16)
            wqk_r = wqk_d.rearrange("(kc p) d -> p kc d", p=P)
            nc.sync.dma_start(wqk_sb[:, 0:3, 0:P], wqk_r[:, 0:3, 0:P])
            nc.sync.dma_start(wqk_sb[:, 3:6, 0:P], wqk_r[:, 3:6, 0:P])
            bqk_sb = cp.tile([P, HPC], F32)
            nc.sync.dma_start(bqk_sb, bqk_d[:])
            nc.sync.dma_start(wqk_sb[:, :, P : 2 * P], wqk_r[:, :, P : 2 * P])
            nc.sync.dma_start(
                wqk_sb[:, :, 2 * P : 3 * P], wqk_r[:, :, 2 * P : 3 * P]
            )

            xt_sb = cp.tile([P, KC, S], BF16)
            xt_r = xt_d.rearrange("(kc p) t -> p kc t", p=P)
            for kc in range(KC):
                nc.sync.dma_start(xt_sb[:, kc, 0:512], xt_r[:, kc, 0:512])
            for kc in range(KC):
                nc.sync.dma_start(xt_sb[:, kc, ts(3, 512)], xt_r[:, kc, ts(3, 512)])
            for ic in (2, 1):
                for kc in range(KC):
                    nc.gpsimd.dma_start(
                        xt_sb[:, kc, ts(ic, 512)], xt_r[:, kc, ts(ic, 512)]
                    )

            wv_sb = cp.tile([P, KC, DV], BF16)
            wv_r = wv_d.rearrange("(kc p) d -> p kc d", p=P)
            nc.sync.dma_start(wv_sb, wv_r)
            mask_sb = cp.tile([P, 4, 2, 512], BF16)
            for k in range(4):
                nc.sync.dma_start(mask_sb[:, k, :, :], mask_d[:, k, :, :])
            wos_sb = cp.tile([HD, HPC, D], BF16)
            nc.sync.dma_start(wos_sb, wos_d[:])

            # V with a ones column (col HD) for the softmax denominator
            v1 = cp.tile([P, NIO, HPC, HD + 1], BF16)
            nc.gpsimd.memset(v1, 1.0)

            # pair-stacked Q^T/K^T for heads 0,1; separate tiles for head 2
            qTp = cp.tile([P, S], BF16)
            klp = cp.tile([P, S], BF16)
            qT2 = cp.tile([HD, S], BF16)
            kl2 = cp.tile([HD, S], BF16)
            ctxT = cp.tile([HD, HPC, S], BF16)

            # ---- PE warm-up: (128,128)-mode matmuls ----
            warm_ps = av_p.tile([P, 512], F32, tag="av", name="warm")
            for w in range(9):
                nc.tensor.matmul(
                    warm_ps[0 : HD + 1, :],
                    lhsT=warm_src[:, 0:65],
                    rhs=warm_src[:, 0:512],
                    start=True,
                    stop=(w == 8),
                )
            warm_sb = zr_p.tile([HD + 1, 512], F32, tag="warm", name="warmsb")
            nc.vector.tensor_copy(warm_sb, warm_ps[0 : HD + 1, :])
            nc.sync.dma_start(scratch_d[:], warm_sb)

            carry = {}   # ic -> (pair ex tiles, h2 ex tiles)

            def proj_slice(ic, sl):
                # wqk slices: 0 -> [K_h0|K_h1], 1 -> [K_h2|Q_h2],
                #             2 -> [Q_h0|Q_h1]
                isl = ts(ic, 512)
                ps = proj_p.tile([P, 512], F32, tag="proj")
                for kc in range(KC):
                    nc.tensor.matmul(
                        ps,
                        lhsT=wqk_sb[:, kc, ts(sl, P)],
                        rhs=xt_sb[:, kc, isl],
                        start=(kc == 0),
                        stop=(kc == KC - 1),
                    )
                if sl == 0:
                    nc.vector.tensor_scalar(
                        klp[:, isl], ps, bqk_sb[:, 0:1], None, ADD,
                    )
                elif sl == 1:
                    nc.vector.tensor_scalar(
                        kl2[:, isl], ps[0:HD, :], bqk_sb[0:HD, 1:2], None, ADD,
                    )
                    nc.vector.tensor_scalar(
                        qT2[:, isl], ps[HD:P, :], bqk_sb[HD:P, 1:2], None, ADD,
                    )
                else:
                    nc.vector.tensor_scalar(
                        qTp[:, isl], ps, bqk_sb[:, 2:3], None, ADD,
                    )

            def trim_of(jc, ic):
                koff = jc - 4 * ic
                return P * koff if koff > 0 else 0

            def sc_group_pair(ic, jc):
                t = trim_of(jc, ic)
                koff = jc - 4 * ic
                sc = sc_p.tile([P, 2, 512], F32, tag="sc", name=f"sp{ic}_{jc}")
                for h in range(2):
                    hsl = ts(h, HD)
                    nc.tensor.matmul(
                        sc[:, h, t:],
                        lhsT=klp[hsl, ts(jc, P)],
                        rhs=qTp[hsl, ic * 512 + t : (ic + 1) * 512],
                        start=True,
                        stop=True,
                    )
                ex = exp_p.tile([P, 2, 512], BF16, tag="ex", name=f"xp{ic}_{jc}")
                nc.scalar.activation(ex[:, :, t:], sc[:, :, t:], AF.Exp)
                if koff >= 0:
                    nc.gpsimd.tensor_tensor(
                        ex[:, :, t:], ex[:, :, t:],
                        mask_sb[:, koff, :, t:], MUL,
                    )
                carry[ic][0].append(ex)

            def sc_group_h2(ic, jb):
                sc = sc_p.tile([P, 2, 512], F32, tag="sc", name=f"s2_{ic}_{jb}")
                for k in range(2):
                    jc = jb + k
                    t = trim_of(jc, ic)
                    nc.tensor.matmul(
                        sc[:, k, t:],
                        lhsT=kl2[:, ts(jc, P)],
                        rhs=qT2[:, ic * 512 + t : (ic + 1) * 512],
                        start=True,
                        stop=True,
                    )
                ex = exp_p.tile([P, 2, 512], BF16, tag="ex", name=f"x2_{ic}_{jb}")
                koff = jb - 4 * ic
                if koff >= 0 and trim_of(jb + 1, ic) > 0:
                    for k in range(2):
                        t = trim_of(jb + k, ic)
                        nc.scalar.activation(ex[:, k, t:], sc[:, k, t:], AF.Exp)
                        nc.gpsimd.tensor_tensor(
                            ex[:, k, t:], ex[:, k, t:],
                            mask_sb[:, koff + k, 0, t:], MUL,
                        )
                else:
                    nc.scalar.activation(ex, sc, AF.Exp)
                    if koff >= 0:
                        nc.gpsimd.tensor_tensor(
                            ex, ex, mask_sb[:, koff : koff + 2, 0, :], MUL,
                        )
                carry[ic][1].append(ex)

            # z chains: per head group, Z rows -> [64,*] via scatter DMA,
            # cheap partition-parallel reciprocal ([64, 8*nh]: cost is
            # column-serial), reshape back to DRAM with the same AP iteration
            # convention (so the round trip is order-exact), then broadcast.
            # Reshape hops dispatch from DVE (zero dispatch-wait); the
            # broadcast from Sync (has a data wait; Sync is idle mid-run).
            zbs = {}    # (ic, h) -> (zb tile, column index)

            def make_zchain(ic, h, state):
                def go():
                    zrow = zr_p.tile([1, 512], F32, tag="zrow",
                                     name=f"zw{ic}{h}")
                    nc.vector.tensor_copy(zrow, state[h][HD : HD + 1, :])
                    zs = zr_p.tile([HD, 8], F32, tag="zs", name=f"zs{ic}{h}")
                    nc.gpsimd.dma_start(zs, zrow)
                    zrs = zr_p.tile([HD, 8], F32, tag="zc", name=f"zc{ic}{h}")
                    nc.vector.reciprocal(zrs, zs)
                    zd = zd_p.tile([1, 512], F32, tag="zd", name=f"zd{ic}{h}")
                    nc.gpsimd.dma_start(zd, zrs)
                    zb = zr_p.tile([HD, 512], F32, tag="zb", name=f"zb{ic}{h}")
                    nc.sync.dma_start(zb, zd[:].to_broadcast((HD, 512)))
                    zbs[(ic, h)] = zb
                return go

            def make_mult(ic, h, state):
                def go():
                    avt = state.pop(h)
                    nc.vector.tensor_tensor(
                        ctxT[:, h, ts(ic, 512)], avt[0:HD, :],
                        zbs.pop((ic, h)), MUL,
                    )
                return go

            def av_stream(ic):
                # mm batches + fin, with each head's mult deferred one head
                n_j = 4 * ic + 4
                state = {}
                work = []

                def mk_mm(h, j0, j1):
                    def go():
                        if h not in state:
                            if ic == 0 and h == 2:
                                t_ = sc_p.tile([P, 2, 512], F32, tag="sc",
                                               name=f"av{ic}{h}")
                                state[h] = t_[:, 0, :]
                            else:
                                state[h] = av_p.tile(
                                    [P, 512], F32, tag="av", name=f"av{ic}{h}"
                                )
                        avt = state[h]
                        exs, exs2 = carry[ic]
                        for jc in range(j0, j1):
                            t = trim_of(jc, ic)
                            exap = (exs[jc][:, h, t:] if h < 2
                                    else exs2[jc // 2][:, jc % 2, t:])
                            nc.tensor.matmul(
                                avt[0 : HD + 1, t:],
                                lhsT=v1[:, jc, h, :],
                                rhs=exap,
                                start=(jc == 0),
                                stop=(jc == n_j - 1),
                            )
                    return go

                if ic == 0:
                    # tail chunk: h2's AV accumulates in a free sc-pool bank;
                    # all AV matmuls run back-to-back, then the three z-chains
                    # launch together so their DMA latencies overlap
                    for h in range(HPC):
                        for j0 in range(0, n_j, 4):
                            work.append(mk_mm(h, j0, min(j0 + 4, n_j)))
                    for h in range(HPC):
                        work.append(make_zchain(ic, h, state))
                    for h in range(HPC):
                        work.append(make_mult(ic, h, state))
                    return work
                for h in range(HPC):
                    for j0 in range(0, n_j, 4):
                        work.append(mk_mm(h, j0, min(j0 + 4, n_j)))
                    work.append(make_zchain(ic, h, state))
                    if h >= 1:
                        work.append(make_mult(ic, h - 1, state))
                work.append(make_mult(ic, HPC - 1, state))
                return work

            out_r = out_d.rearrange("(io p) d -> p io d", p=P)
            opair = {}

            def oproj_units(ic):
                # units in io pairs sharing one o_sb tile and one output DMA
                units = []
                for io4 in range(4):
                    io = ic * 4 + io4

                    def unit(io=io):
                        if io % 2 == 0:
                            opair[io // 2] = op.tile(
                                [P, 2, D], BF16, tag="osb", name=f"ou{io}"
                            )
                        o_sb = opair[io // 2]
                        for ot, ow in ((0, 512), (1, 256)):
                            ps = proj_p.tile([P, 512], F32, tag="proj")
                            pso = ps[:, :ow]
                            for h in range(HPC):
                                nc.tensor.matmul(
                                    pso,
                                    lhsT=ctxT[:, h, ts(io, P)],
                                    rhs=wos_sb[:, h, ot * 512 : ot * 512 + ow],
                                    start=(h == 0),
                                    stop=(h == HPC - 1),
                                )
                            nc.vector.tensor_copy(
                                o_sb[:, io % 2, ot * 512 : ot * 512 + ow], pso
                            )
                        if io % 2 == 1:
                            nc.sync.dma_start(
                                out_r[:, io - 1 : io + 1, :],
                                opair.pop(io // 2),
                            )

                    units.append(unit)
                return units

            def v_units(ic):
                units = []
                for io in range(ic * 4, ic * 4 + 4):
                    def unit(io=io):
                        ps = proj_p.tile([P, 512], F32, tag="proj")
                        psv = ps[:, :DV]
                        c0 = io * P
                        for kc in range(KC):
                            nc.tensor.matmul(
                                psv,
                                lhsT=xt_sb[:, kc, c0 : c0 + P],
                                rhs=wv_sb[:, kc, :],
                                start=(kc == 0),
                                stop=(kc == KC - 1),
                            )
                        nc.vector.tensor_copy(
                            v1[:, io, :, 0:HD],
                            psv.rearrange("p (h e) -> p h e", e=HD),
                        )
                    units.append(unit)
                return units

            # dummy matmuls keep PE activity above HAM's re-throttle window
            # through the sparse tail (they write scratch psum, never read)
            def dummy64():
                d = sc_p.tile([P, 2, 512], F32, tag="sc", name="dm64")
                nc.tensor.matmul(
                    d[0:65, 0, :], lhsT=warm_src[0:HD, 0:65],
                    rhs=warm_src[0:HD, 0:512], start=True, stop=True,
                )
                return d

            def dummy128():
                d = proj_p.tile([P, 512], F32, tag="proj", name="dm128")
                nc.tensor.matmul(
                    d[0:65, :], lhsT=warm_src[:, 0:65],
                    rhs=warm_src[:, 0:512], start=True, stop=True,
                )
                return d

            def run_phase(ic, work128, work64=(), keep_warm=False,
                          h2_delay=False):
                # Emit scores for chunk ic in jb-slots; after each slot emit a
                # proportional share of 64-mode fillers (oproj) and 128-mode
                # work (V proj or AV of the larger chunk).
                carry[ic] = ([], [])
                n_j = 4 * ic + 4
                work64 = list(work64)
                work128 = list(work128)
                nslots = n_j // 2
                d64 = d128 = 0
                pend_h2 = None
                for s, jb in enumerate(range(0, n_j, 2)):
                    sc_group_pair(ic, jb)
                    sc_group_pair(ic, jb + 1)
                    if h2_delay:
                        if pend_h2 is not None:
                            sc_group_h2(ic, pend_h2)
                        pend_h2 = jb
                    else:
                        sc_group_h2(ic, jb)
                    w = len(work64) * (s + 1) // nslots
                    while d64 < w:
                        work64[d64]()
                        d64 += 1
                    if keep_warm:
                        dummy64()
                    w = len(work128) * (s + 1) // nslots
                    while d128 < w:
                        work128[d128]()
                        d128 += 1
                    if keep_warm:
                        dummy128()
                if pend_h2 is not None:
                    sc_group_h2(ic, pend_h2)

            # ---------------- main schedule ----------------
            # K(0) + Q(3) first so scores(3) -- and the ACT exp stream --
            # start as early as possible; the remaining K slices drip in as
            # 128-mode fillers just ahead of the key chunks that need them.
            # V/Q projections of the small chunks are pushed late to densify
            # the back half of the kernel (keeps HAM un-throttled).
            proj_slice(0, 0)   # K-pair(0): pair scores jc 0-3
            proj_slice(3, 2)   # Q-pair(3)
            kdrip = [lambda: proj_slice(0, 1), lambda: proj_slice(3, 1)]
            for ic in (1, 2):
                kdrip.append(lambda ic=ic: proj_slice(ic, 0))
                kdrip.append(lambda ic=ic: proj_slice(ic, 1))
            kdrip.append(lambda: proj_slice(3, 0))
            run_phase(3, kdrip + [lambda: proj_slice(2, 2)]
                      + v_units(3) + v_units(2) + v_units(1) + v_units(0),
                      h2_delay=True)
            run_phase(2, av_stream(3) + [lambda: proj_slice(1, 2)])
            run_phase(1, av_stream(2) + [lambda: proj_slice(0, 2)],
                      oproj_units(3), keep_warm=True)
            run_phase(0, av_stream(1), oproj_units(2), keep_warm=True)
            # oproj(1) split around av(0): the first half runs before (its
            # deps resolve quickly); the second half fills the PE while the
            # ic0 z-chain DMA latencies resolve
            ou1 = oproj_units(1)
            for u in ou1[:2]:
                u()
                dummy64()
            d128 = None
            for i, item in enumerate(av_stream(0)):
                item()
                if i % 2 == 1:
                    d128 = dummy128()
            d64 = None
            for u in ou1[2:] + oproj_units(0):
                u()
                d64 = dummy64()
            drain = zr_p.tile([HD, 512], F32, tag="ddr", name="ddrain")
            if d64 is not None:
                nc.vector.tensor_copy(drain, d64[0:HD, 0, :])
            if d128 is not None:
                nc.vector.tensor_copy(drain, d128[0:HD, :])

    _split_excess_waits(nc)
    return nc


_NC = None


def _get_nc():
    global _NC
    if _NC is None:
        _NC = _build_module()
    return _NC


def _make_mask():
    p = np.arange(P)[:, None]
    f = np.arange(512)[None, :]
    m = np.empty((P, 4, 2, 512), np.float32)
    for k in range(4):
        m[:, k, 0, :] = (p <= f - P * k).astype(np.float32)
        m[:, k, 1, :] = m[:, k, 0, :]
    return m.astype(NPBF16)


def _build_in_maps(x, wq, bq, wk, bk, wv, bv, wo):
    scale = 1.0 / np.sqrt(HD)
    mask = _make_mask()
    in_maps = []
    for core in range(8):
        b = core // 4
        h0 = (core % 4) * HPC

        # pair-packed slices: [K_h0|K_h1], [K_h2|Q_h2], [Q_h0|Q_h1]
        wqk = np.empty((D, DQK), np.float32)
        bqk = np.empty((P, HPC), np.float32)
        cs = [slice((h0 + i) * HD, (h0 + i + 1) * HD) for i in range(HPC)]
        wqk[:, 0:HD] = wk[:, cs[0]]
        wqk[:, HD:P] = wk[:, cs[1]]
        wqk[:, P : P + HD] = wk[:, cs[2]]
        wqk[:, P + HD : 2 * P] = wq[:, cs[2]] * scale
        wqk[:, 2 * P : 2 * P + HD] = wq[:, cs[0]] * scale
        wqk[:, 2 * P + HD : 3 * P] = wq[:, cs[1]] * scale
        bqk[0:HD, 0] = bk[cs[0]]
        bqk[HD:P, 0] = bk[cs[1]]
        bqk[0:HD, 1] = bk[cs[2]]
        bqk[HD:P, 1] = bq[cs[2]] * scale
        bqk[0:HD, 2] = bq[cs[0]] * scale
        bqk[HD:P, 2] = bq[cs[1]] * scale

        vcols = slice(h0 * HD, (h0 + HPC) * HD)
        wos = (
            wo[vcols, :].reshape(HPC, HD, D).transpose(1, 0, 2)
        )  # [HD, HPC, D]

        in_maps.append(
            {
                "xt": np.ascontiguousarray(x[b].T).astype(NPBF16),
                "wqk": wqk.astype(NPBF16),
                "bqk": bqk.astype(np.float32),
                "wv": np.ascontiguousarray(wv[:, vcols]).astype(NPBF16),
                "wos": np.ascontiguousarray(wos).astype(NPBF16),
                "mask": mask,
            }
        )
    return in_maps


def kernel(x, wq, bq, wk, bk, wv, bv, wo, bo):
    x = np.asarray(x, np.float32)
    wq = np.asarray(wq, np.float32)
    bq = np.asarray(bq, np.float32)
    wk = np.asarray(wk, np.float32)
    bk = np.asarray(bk, np.float32)
    wv = np.asarray(wv, np.float32)
    bv = np.asarray(bv, np.float32)
    wo = np.asarray(wo, np.float32)
    bo = np.asarray(bo, np.float32)

    in_maps = _build_in_maps(x, wq, bq, wk, bk, wv, bv, wo)
    res = run_bass_kernel_spmd(_get_nc(), in_maps, core_ids=list(range(8)))
    out = np.zeros((B, S, D), np.float32)
    for core in range(8):
        out[core // 4] += np.asarray(res.results[core]["out"], np.float32)
    out += bo + bv @ wo
    return out
